# revision 27
# baseline (speedup 1.0000x reference)
# Trainium2 Bass kernel for a single pre-norm transformer block
# (LN1 -> 6-head causal self-attention -> residual -> LN2 -> 1536-wide relu MLP -> residual).
#
# Sharding: pure data-parallel over batch. B=128 sequences split 16-per-core
# across 8 NeuronCores; weights are replicated; no collectives.
#
# Per-core layout strategy (v2, all-bf16 matmul path):
#   - Activations "feature-major" (FM): [C partitions (3x128 chunks), tokens];
#     chained matmuls need no transposes. LN runs token-major (bn_stats over
#     free dim), normalized tile PE-transposed into FM. LN gamma folded into
#     the bf16 weight conversion; LN beta folded into the next matmul's bias.
#   - All matmul operands bf16 (weights converted on-chip once). PE runs
#     1 cycle/row at ANY free size, so the masked-out half of the second
#     score/attnV matmuls is skipped (narrow [P,384] score layout).
#   - Softmax: transposed scores, no max-subtraction. Denominators come from
#     tiny expT @ ones matmuls into PSUM columns, ONE batched reciprocal per
#     sequence, one PE transpose, and per-head PE row-broadcasts that
#     overwrite the drained attnV psum; DVE multiplies sbuf x psum.
#   - V bias (from LN1 beta) is folded through Wo into bo (bo2 = bo + bv@Wo),
#     so V needs no bias add. Residual pre-adds (x+bo2, o1+b2) run on the
#     otherwise-idle GPSIMD engine, leaving single DVE adds at Wo/FFN out.

import numpy as np

P = 128
B, T, C, H, D = 128, 256, 384, 6, 64
NCORES = 8
B_LOC = B // NCORES          # 16 sequences per core
NTOK = B_LOC * T             # 4096 tokens per core
TS = 2 * T                   # 512-token supertile = 2 sequences
NSUP = NTOK // TS            # 8
CJ = C // P                  # 3 chunks of the 384 model dim
FF = 4 * C                   # 1536
FJ = FF // P                 # 12 chunks of the FFN hidden dim
NTT = TS // P                # 4 token tiles per supertile
EPS = 1e-5
SCALE = D ** (-0.5)

_CACHE = {}


def _pbc(ap_row, parts):
    """Broadcast a [1, ...] AP across `parts` partitions (0-stride partition dim)."""
    import concourse.bass as bass
    return bass.AP(
        tensor=ap_row.tensor,
        offset=ap_row.offset,
        ap=[[0, parts]] + [list(d) for d in ap_row.ap[1:]],
    )


def _build_nc(niter=1):
    import concourse.bass as bass
    import concourse.tile as tile
    from concourse import bacc, mybir
    from concourse.masks import make_identity, make_upper_triangular
    from contextlib import ExitStack

    F32 = mybir.dt.float32
    F32R = mybir.dt.float32r
    BF16 = mybir.dt.bfloat16
    FP8 = mybir.dt.float8e4
    DR = mybir.MatmulPerfMode.DoubleRow

    def rb(dram_ap):
        return dram_ap.bitcast(F32R)

    nc = bacc.Bacc("TRN2", target_bir_lowering=False, debug=False,
                   num_devices=NCORES)

    x_d = nc.dram_tensor("x", [NTOK, C], F32, kind="ExternalInput").ap()
    ln1_g = nc.dram_tensor("ln1_g", [C], F32, kind="ExternalInput").ap()
    ln1_b = nc.dram_tensor("ln1_b", [C], F32, kind="ExternalInput").ap()
    Wq_d = nc.dram_tensor("Wq", [H, C, D], F32, kind="ExternalInput").ap()
    Wk_d = nc.dram_tensor("Wk", [H, C, D], F32, kind="ExternalInput").ap()
    Wv_d = nc.dram_tensor("Wv", [H, C, D], F32, kind="ExternalInput").ap()
    Wo_d = nc.dram_tensor("Wo", [C, C], F32, kind="ExternalInput").ap()
    bo_d = nc.dram_tensor("bo", [C], F32, kind="ExternalInput").ap()
    ln2_g = nc.dram_tensor("ln2_g", [C], F32, kind="ExternalInput").ap()
    ln2_b = nc.dram_tensor("ln2_b", [C], F32, kind="ExternalInput").ap()
    W1_d = nc.dram_tensor("W1", [C, FF], F32, kind="ExternalInput").ap()
    b1_d = nc.dram_tensor("b1", [FF], F32, kind="ExternalInput").ap()
    W2_d = nc.dram_tensor("W2", [FF, C], F32, kind="ExternalInput").ap()
    b2_d = nc.dram_tensor("b2", [C], F32, kind="ExternalInput").ap()
    out_d = nc.dram_tensor("out", [NTOK, C], F32, kind="ExternalOutput").ap()
    bo2_scr = nc.dram_tensor("bo2_scr", [1, C], F32).ap()  # internal scratch

    Exp = mybir.ActivationFunctionType.Exp
    Relu = mybir.ActivationFunctionType.Relu
    Ident = mybir.ActivationFunctionType.Identity
    I32 = mybir.dt.int32
    ADD = mybir.AluOpType.add
    MULT = mybir.AluOpType.mult
    SHR = mybir.AluOpType.logical_shift_right

    with tile.TileContext(nc) as tc, ExitStack() as ctx:
        consts = ctx.enter_context(tc.tile_pool(name="consts", bufs=1))
        wpool = ctx.enter_context(tc.tile_pool(name="weights", bufs=1))
        xpool = ctx.enter_context(tc.tile_pool(name="xln", bufs=8))
        ps_big = ctx.enter_context(tc.tile_pool(name="psbig", bufs=3, space="PSUM"))
        ps_tr = ctx.enter_context(tc.tile_pool(name="pstr", bufs=2, space="PSUM"))
        ps_dn = ctx.enter_context(tc.tile_pool(name="psdn", bufs=1, space="PSUM"))
        ps_at = ctx.enter_context(tc.tile_pool(name="psat", bufs=2, space="PSUM"))

        def load_x(s):
            tok0 = s * TS
            x_ts = []
            for ti in range(NTT):
                x_t = xpool.tile([P, C], F32, tag="x")
                nc.sync.dma_start(
                    x_t[:], x_d[tok0 + ti * P: tok0 + (ti + 1) * P, :])
                x_ts.append(x_t)
            return x_ts

        # ---------------- constants ----------------
        ident = consts.tile([P, P], F32, tag="ident")
        make_identity(nc, ident[:])
        ident_bf = consts.tile([P, P], BF16, tag="identbf")
        nc.gpsimd.tensor_copy(ident_bf[:], ident[:])
        # causal mask for the narrow transposed-scores layout: same upper
        # triangle serves block [s 0:128 x t 0:128] and [s 128:256 x t 128:256]
        mask128 = consts.tile([P, P], BF16, tag="mask")
        make_upper_triangular(nc, mask128[:])
        ones_col = consts.tile([P, 1], BF16, tag="ones_col")
        nc.gpsimd.memset(ones_col[:], 1.0)
        # head-pair indicator for the recip row-broadcast: out row p gets
        # recip row 0 (head even) for p<64, row 1 (head odd) for p>=64.
        # Built as a [P,2] column tile (partition-base rules) and transposed.
        ind_col = consts.tile([P, 2], BF16, tag="ind_col")
        nc.gpsimd.memset(ind_col[:], 0.0)
        nc.gpsimd.memset(ind_col[0:D, 0:1], 1.0)
        nc.gpsimd.memset(ind_col[D:P, 1:2], 1.0)
        ind2 = consts.tile([2, P], BF16, tag="ind2")
        ps_ind = ps_tr.tile([2, P], BF16, tag="tr")
        nc.tensor.transpose(ps_ind[:], ind_col[:], ident_bf[:])
        nc.vector.tensor_copy(ind2[:], ps_ind[:])

        def load_vec_fm(vec_ap, n_chunks, tag, dt=F32):
            t = consts.tile([P, n_chunks], dt, tag=tag)
            src_ap = vec_ap.rearrange("(j p) -> p j", p=P)
            nc.sync.dma_start(t[:], src_ap.bitcast(dt) if dt is F32R else src_ap)
            return t

        g1 = load_vec_fm(ln1_g, CJ, "g1")
        g2 = load_vec_fm(ln2_g, CJ, "g2")
        b1f = load_vec_fm(b1_d, FJ, "b1f")

        def load_beta_padded(vec_ap, tag):
            # [P, CJ+1] f32r: fp32r matmuls need even moving-operand counts,
            # so beta is used as a 2-wide rhs slice; the pad column's product
            # lands in an ignored psum column.
            t = consts.tile([P, CJ + 1], F32R, tag=tag)
            nc.sync.dma_start(t[:, 0:CJ],
                              vec_ap.rearrange("(j p) -> p j", p=P).bitcast(F32R))
            nc.vector.tensor_copy(t[:, CJ:CJ + 1], t[:, 0:1])
            return t

        b1ln = load_beta_padded(ln1_b, "b1ln")
        b2ln = load_beta_padded(ln2_b, "b2ln")

        b2_bc = consts.tile([P, C], F32, tag="b2_bc")
        nc.sync.dma_start(b2_bc[:], _pbc(b2_d[None, :], P))

        # ---------------- weights: fp32 staging (pool released after
        # conversion so its SBUF range is reusable by activation pools) ----
        wstage_cm = tc.tile_pool(name="wstage", bufs=1)
        wstage = wstage_cm.__enter__()

        def load_w_hcd(w_ap, tag):
            t = wstage.tile([P, CJ, H, D], F32R, tag=tag)
            for h in range(H):
                nc.sync.dma_start(t[:, :, h, :],
                                  rb(w_ap[h].rearrange("(j p) d -> p j d", p=P)))
            return t[:].rearrange("p j h d -> p j (h d)")  # [P, CJ, C] view

        wq_st = load_w_hcd(Wq_d, "wq")
        wk_st = load_w_hcd(Wk_d, "wk")
        wv_st = load_w_hcd(Wv_d, "wv")
        wo_st = wstage.tile([P, CJ, C], F32R, tag="wo")
        nc.sync.dma_start(wo_st[:], rb(Wo_d.rearrange("(j p) f -> p j f", p=P)))

        # Fused biases from folding LN beta into the next matmul: bias = beta @ W.
        def proj_bias_fm(w_view, nf_chunks, beta_fm, tag, add_to=None):
            bt = consts.tile([P, nf_chunks], F32, tag=tag)
            for f in range(nf_chunks):
                ps = ps_tr.tile([P, 2], F32, tag="tr")
                for j in range(CJ):
                    nc.tensor.matmul(ps[:], lhsT=w_view[:, j, f * P:(f + 1) * P],
                                     rhs=beta_fm[:, j:j + 2],
                                     start=(j == 0), stop=(j == CJ - 1))
                if add_to is None:
                    nc.vector.tensor_copy(bt[:, f:f + 1], ps[:, 0:1])
                else:
                    nc.vector.tensor_tensor(bt[:, f:f + 1], ps[:, 0:1],
                                            add_to[:, f:f + 1], op=ADD)
            return bt

        bq = proj_bias_fm(wq_st, CJ, b1ln, "bq")
        MAX = mybir.AluOpType.max
        bk = proj_bias_fm(wk_st, CJ, b1ln, "bk")
        # V bias folded through Wo into bo: bo2 = bo + (ln1_b @ Wv) @ Wo.
        bv_fm = consts.tile([P, CJ + 1], F32R, tag="bv_fm")
        for f in range(CJ):
            ps = ps_tr.tile([P, 2], F32, tag="tr")
            for j in range(CJ):
                nc.tensor.matmul(ps[:], lhsT=wv_st[:, j, f * P:(f + 1) * P],
                                 rhs=b1ln[:, j:j + 2],
                                 start=(j == 0), stop=(j == CJ - 1))
            nc.vector.tensor_copy(bv_fm[:, f:f + 1], ps[:, 0:1])
        bo_row = consts.tile([1, C], F32, tag="bo_row")
        nc.sync.dma_start(bo_row[:], bo_d[None, :])
        ps_bv = ps_big.tile([1, C], F32, tag="big")
        for j in range(CJ):
            nc.tensor.matmul(ps_bv[:], lhsT=bv_fm[:, j:j + 1], rhs=wo_st[:, j, :],
                             start=(j == 0), stop=(j == CJ - 1))
        bo2_row = consts.tile([1, C], F32, tag="bo2_row")
        nc.vector.tensor_tensor(bo2_row[:], ps_bv[:], bo_row[:], op=ADD)
        nc.sync.dma_start(bo2_scr[:], bo2_row[:])
        bo2_bc = consts.tile([P, C], F32, tag="bo2_bc")
        nc.sync.dma_start(bo2_bc[:], _pbc(bo2_scr, P))

        # attention-path weights converted first (gamma folded into the copy)
        wq = wpool.tile([P, CJ, C], BF16, tag="wqb")
        wk = wpool.tile([P, CJ, C], BF16, tag="wkb")
        wv = wpool.tile([P, CJ, C], BF16, tag="wvb")
        wo = wpool.tile([P, CJ, C], BF16, tag="wob")
        w1 = wpool.tile([P, CJ, FF], FP8, tag="w1b")
        w2 = wpool.tile([P, FJ, C], BF16, tag="w2b")
        def conv(dst, src_v, scale_t, j, eng_i):
            eng = (nc.vector, nc.scalar, nc.gpsimd)[eng_i % 3]
            if eng is nc.scalar:
                if scale_t is None:
                    nc.scalar.activation(dst, src_v, Ident)
                else:
                    nc.scalar.activation(dst, src_v, Ident,
                                         scale=scale_t[:, j:j + 1])
            elif scale_t is None:
                eng.tensor_copy(dst, src_v)
            else:
                eng.tensor_scalar_mul(dst, src_v, scale_t[:, j:j + 1])

        for j in range(CJ):
            conv(wv[:, j], wv_st[:, j], g1, j, j)
            conv(wq[:, j], wq_st[:, j], g1, j, j + 1)
            conv(wk[:, j], wk_st[:, j], g1, j, j + 2)
            conv(wo[:, j], wo_st[:, j], None, j, j)

        # FFN weights stream + convert (outside the main loop)
        w1_st = wstage.tile([P, CJ, FF], F32R, tag="w1")
        nc.sync.dma_start(w1_st[:], rb(W1_d.rearrange("(j p) f -> p j f", p=P)))
        w2_st = wstage.tile([P, FJ, C], F32R, tag="w2")
        nc.sync.dma_start(w2_st[:], rb(W2_d.rearrange("(j p) f -> p j f", p=P)))
        b1p = proj_bias_fm(w1_st, FJ, b2ln, "b1p", add_to=b1f)  # b1 + ln2_b @ W1
        b1p16 = consts.tile([P, FJ], F32, tag="b1p16")
        nc.vector.tensor_scalar(b1p16[:], b1p[:], 16.0, None, op0=MULT)
        g2s = consts.tile([P, CJ], F32, tag="g2s")
        nc.vector.tensor_scalar(g2s[:], g2[:], 16.0, None, op0=MULT)
        for j in range(CJ):
            conv(w1[:, j], w1_st[:, j], g2s, j, j)
        # z is stored as 16*relu(.): fold the 1/16 into W2
        s16 = consts.tile([P, 1], F32, tag="s16")
        nc.vector.memset(s16[:], 1.0 / 16.0)
        for j in range(FJ):
            conv(w2[:, j], w2_st[:, j], s16, 0, j)

        wstage_cm.__exit__(None, None, None)

        # ---------------- layernorm helpers ----------------
        spool = ctx.enter_context(tc.tile_pool(name="stats", bufs=6))
        ynpool = ctx.enter_context(tc.tile_pool(name="yn", bufs=8))

        def ln_stats(src_tiles):
            """Token-major mean/rstd for NTT tiles (DVE chain)."""
            mv4 = spool.tile([P, NTT, 2], F32, tag="mv")
            rstd4 = spool.tile([P, NTT], F32, tag="rstd")
            for ti in range(NTT):
                st = spool.tile([P, 6], F32, tag="bn")
                nc.vector.bn_stats(st[:], src_tiles[ti][:])
                nc.vector.bn_aggr(mv4[:, ti, :], st[:])
            # rstd = rsqrt(var + eps): int32 seed + 2 Newton steps (no tables)
            veps = spool.tile([P, NTT], F32, tag="veps")
            nc.vector.tensor_scalar_add(veps[:], mv4[:, :, 1], EPS)
            iv = spool.tile([P, NTT], I32, tag="ivh")
            nc.vector.tensor_scalar(iv[:], veps[:].bitcast(I32), 1, None, op0=SHR)
            nc.vector.tensor_scalar(iv[:], iv[:], -1, 0x5F3759DF, op0=MULT, op1=ADD)
            tn = spool.tile([P, NTT], F32, tag="tnh")
            yv = iv[:].bitcast(F32)
            for it in range(2):
                nc.vector.tensor_tensor(tn[:], yv, yv, op=MULT)
                nc.vector.scalar_tensor_tensor(tn[:], tn[:], -0.5, veps[:],
                                               op0=MULT, op1=MULT)
                nc.vector.scalar_tensor_tensor(yv, tn[:], 1.5, yv,
                                               op0=ADD, op1=MULT)
            nc.vector.tensor_copy(rstd4[:], yv)
            nbias = spool.tile([P, NTT], F32, tag="nb")
            nc.vector.scalar_tensor_tensor(nbias[:], mv4[:, :, 0], -1.0,
                                           rstd4[:], op0=MULT, op1=MULT)
            return rstd4, nbias

        def ln_apply(src_tiles, stats, dst_fm):
            """Normalize token-major (Act) + PE transpose to FM + DVE copy."""
            rstd4, nbias = stats
            for ti in range(NTT):
                yn = ynpool.tile([P, C], BF16, tag="yn")
                nc.scalar.activation(yn[:], src_tiles[ti][:], Ident,
                                     bias=nbias[:, ti:ti + 1],
                                     scale=rstd4[:, ti:ti + 1])
                pst = ps_tr.tile([P, C], BF16, tag="tr")
                for j in range(CJ):
                    nc.tensor.transpose(pst[:, j * P:(j + 1) * P],
                                        yn[:, j * P:(j + 1) * P], ident_bf[:])
                nc.vector.tensor_copy(
                    dst_fm[:, :, ti * P:(ti + 1) * P],
                    pst[:].rearrange("p (j t) -> p j t", j=CJ))

        # ---------------- first supertile front ----------------
        hpool = ctx.enter_context(tc.tile_pool(name="hfm", bufs=2))
        h2pool = ctx.enter_context(tc.tile_pool(name="h2fm", bufs=2))
        qkpool = ctx.enter_context(tc.tile_pool(name="qk", bufs=3))
        vpool = ctx.enter_context(tc.tile_pool(name="vton", bufs=8))

        def ln1_full(x_ts):
            h_fm = hpool.tile([P, CJ, TS], BF16, tag="hfm")
            ln_apply(x_ts, ln_stats(x_ts), h_fm)
            return h_fm

        def qkv_phase(h_fm):
            q_fm = qkpool.tile([P, CJ, TS], BF16, tag="qk")
            k_fm = qkpool.tile([P, CJ, TS], BF16, tag="qk")
            for wt, bt, dst in ((wq, bq, q_fm), (wk, bk, k_fm)):
                for f in range(CJ):
                    ps = ps_big.tile([P, TS], F32, tag="big")
                    for j in range(CJ):
                        nc.tensor.matmul(
                            ps[:], lhsT=wt[:, j, f * P:(f + 1) * P],
                            rhs=h_fm[:, j, :],
                            start=(j == 0), stop=(j == CJ - 1))
                    nc.scalar.activation(dst[:, f, :], ps[:], Ident,
                                         bias=bt[:, f:f + 1])
            v_ts = []
            for ti in range(NTT):
                ps = ps_big.tile([P, C], F32, tag="big")
                for j in range(CJ):
                    nc.tensor.matmul(
                        ps[:], lhsT=h_fm[:, j, ti * P:(ti + 1) * P],
                        rhs=wv[:, j, :],
                        start=(j == 0), stop=(j == CJ - 1))
                v_t = vpool.tile([P, C], BF16, tag="v")
                nc.scalar.activation(v_t[:], ps[:], Ident)
                v_ts.append(v_t)
            return q_fm, k_fm, v_ts

        xbpool = ctx.enter_context(tc.tile_pool(name="xbo", bufs=6))
        o1pool = ctx.enter_context(tc.tile_pool(name="o1res", bufs=6))
        obpool = ctx.enter_context(tc.tile_pool(name="o1b2", bufs=6))
        apool = ctx.enter_context(tc.tile_pool(name="attnfm", bufs=2))
        epool = ctx.enter_context(tc.tile_pool(name="expT", bufs=6))
        arpool = ctx.enter_context(tc.tile_pool(name="attnraw", bufs=6))
        zpool = ctx.enter_context(tc.tile_pool(name="zfm", bufs=1))
        ypool = ctx.enter_context(tc.tile_pool(name="yout", bufs=3))

        # ---------------- main phases ----------------
        def attn_scores(q_fm, k_fm):
            """Scores + exp + masks for all 12 (seq, head) blocks. Emitted a
            phase early so Act streams exps during the previous FFN."""
            exps = []
            for seq in range(2):
                t0 = seq * T
                seq_exps = []
                for h in range(H):
                    hp, hh = h // 2, h % 2
                    pr = slice(hh * D, (hh + 1) * D)
                    # narrow scores: cols 0:256 = [s 0:128] x [t 0:256],
                    # cols 256:384 = [s 128:256] x [t 128:256]
                    ps_sc = ps_big.tile([P, 3 * P], F32, tag="big")
                    nc.tensor.matmul(ps_sc[:, 0:T],
                                     lhsT=k_fm[pr, hp, t0:t0 + P],
                                     rhs=q_fm[pr, hp, t0:t0 + T],
                                     start=True, stop=True)
                    nc.tensor.matmul(ps_sc[:, T:T + P],
                                     lhsT=k_fm[pr, hp, t0 + P:t0 + T],
                                     rhs=q_fm[pr, hp, t0 + P:t0 + T],
                                     start=True, stop=True)
                    expT = epool.tile([P, 3 * P], BF16, tag="e")
                    nc.scalar.activation(expT[:], ps_sc[:], Exp, scale=SCALE)
                    nc.vector.tensor_tensor(expT[:, 0:P], expT[:, 0:P],
                                            mask128[:], op=MULT)
                    nc.gpsimd.tensor_tensor(expT[:, T:T + P], expT[:, T:T + P],
                                            mask128[:], op=MULT)
                    seq_exps.append(expT)
                exps.append(seq_exps)
            return exps

        def attention_phase(exps_2, v_ts):
            attn_fm = apool.tile([P, CJ, TS], BF16, tag="attn")
            for seq in range(2):
                t0 = seq * T
                v0, v1 = v_ts[2 * seq], v_ts[2 * seq + 1]
                exps = exps_2[seq]
                dn = ps_dn.tile([P, 2 * H], F32, tag="dn")
                for h in range(H):
                    expT = exps[h]
                    nc.tensor.matmul(dn[:, h:h + 1], lhsT=expT[:, 0:P],
                                     rhs=ones_col[:], start=True, stop=True)
                    nc.tensor.matmul(dn[:, H + h:H + h + 1],
                                     lhsT=expT[:, P:T], rhs=ones_col[:],
                                     start=True, stop=False)
                    nc.tensor.matmul(dn[:, H + h:H + h + 1],
                                     lhsT=expT[:, T:T + P], rhs=ones_col[:],
                                     start=False, stop=True)
                pas, ars = [], []
                for hp in range(CJ):
                    ps_a = ps_at.tile([P, T], F32, tag="at")
                    for hh in range(2):
                        h = 2 * hp + hh
                        po = slice(hh * D, (hh + 1) * D)
                        nc.tensor.matmul(ps_a[po, 0:T],
                                         lhsT=v0[:, h * D:(h + 1) * D],
                                         rhs=exps[h][:, 0:T],
                                         start=True, stop=False)
                        nc.tensor.matmul(ps_a[po, P:T],
                                         lhsT=v1[:, h * D:(h + 1) * D],
                                         rhs=exps[h][:, T:T + P],
                                         start=False, stop=True)
                    ar = arpool.tile([P, T], BF16, tag="ar")
                    nc.vector.tensor_copy(ar[:], ps_a[:])
                    pas.append(ps_a)
                    ars.append(ar)
                # one reciprocal for all 12 (t-chunk, head) denominators, then
                # head-pair columns -> rows at partitions 0:2 via ident-matmul
                recip = spool.tile([P, 2 * H], BF16, tag="recip")
                with nc.allow_low_precision(reason="bf16 softmax recip"):
                    nc.vector.reciprocal(recip[:], dn[:])
                rows = []
                for m in range(2):
                    ps_row = ps_dn.tile([2, 3 * P], F32, tag="dn")
                    for c in range(3):
                        nc.tensor.matmul(ps_row[0:2, c * P:(c + 1) * P],
                                         lhsT=recip[:, 6 * m + 2 * c:
                                                    6 * m + 2 * c + 2],
                                         rhs=ident_bf[:], start=True, stop=True)
                    row_sb = spool.tile([2, 3 * P], BF16, tag="rrows")
                    nc.vector.tensor_copy(row_sb[:], ps_row[:])
                    rows.append(row_sb)
                for hp in range(CJ):
                    ps_a = pas[hp]
                    # recip row-broadcast overwrites the drained attnV psum:
                    # rows[0] has t-chunk0 pairs, rows[1] has t-chunk1 pairs
                    nc.tensor.matmul(ps_a[:, 0:P], lhsT=ind2[:],
                                     rhs=rows[0][0:2, hp * P:(hp + 1) * P],
                                     start=True, stop=True)
                    nc.tensor.matmul(ps_a[:, P:T], lhsT=ind2[:],
                                     rhs=rows[1][0:2, hp * P:(hp + 1) * P],
                                     start=True, stop=True)
                    nc.vector.tensor_tensor(attn_fm[:, hp, t0:t0 + T],
                                            ars[hp][:], ps_a[:], op=MULT)
            return attn_fm

        def wo_phase(attn_fm, x_ts):
            o1_ts, ob_ts = [], []
            for ti in range(NTT):
                xb = xbpool.tile([P, C], F32, tag="xb")
                nc.gpsimd.tensor_tensor(xb[:], x_ts[ti][:], bo2_bc[:], op=ADD)
                ps = ps_big.tile([P, C], F32, tag="big")
                for j in range(CJ):
                    nc.tensor.matmul(
                        ps[:], lhsT=attn_fm[:, j, ti * P:(ti + 1) * P],
                        rhs=wo[:, j, :],
                        start=(j == 0), stop=(j == CJ - 1))
                o1 = o1pool.tile([P, C], F32, tag="o1")
                nc.vector.tensor_tensor(o1[:], ps[:], xb[:], op=ADD)
                o1_ts.append(o1)
                ob = obpool.tile([P, C], F32, tag="ob")
                nc.gpsimd.tensor_tensor(ob[:], o1[:], b2_bc[:], op=ADD)
                ob_ts.append(ob)
            return o1_ts, ob_ts

        def ffn_phase(h2_fm, ob_ts, tok0):
            z_fm = zpool.tile([P, FJ, TS], BF16, tag="z")
            for f in range(FJ):
                ps = ps_big.tile([P, TS], F32, tag="big")
                nc.tensor.matmul(
                    ps[:], lhsT=w1[:, 0:2, f * P:(f + 1) * P],
                    rhs=h2_fm[:, 0:2, :], perf_mode=DR,
                    start=True, stop=False)
                nc.tensor.matmul(
                    ps[:], lhsT=w1[:, 2, f * P:(f + 1) * P],
                    rhs=h2_fm[:, 2, :],
                    start=False, stop=True)
                if f % 2 == 0:
                    nc.scalar.activation(z_fm[:, f, :], ps[:], Relu,
                                         bias=b1p16[:, f:f + 1])
                else:
                    nc.vector.tensor_scalar(z_fm[:, f, :], ps[:],
                                            b1p16[:, f:f + 1], 0.0,
                                            op0=ADD, op1=MAX)
            for ti in range(NTT):
                ps = ps_big.tile([P, C], F32, tag="big")
                for j in range(FJ):
                    nc.tensor.matmul(
                        ps[:], lhsT=z_fm[:, j, ti * P:(ti + 1) * P],
                        rhs=w2[:, j, :],
                        start=(j == 0), stop=(j == FJ - 1))
                y_t = ypool.tile([P, C], F32, tag="y")
                nc.vector.tensor_tensor(y_t[:], ps[:], ob_ts[ti][:], op=ADD)
                nc.sync.dma_start(
                    out_d[tok0 + ti * P: tok0 + (ti + 1) * P, :], y_t[:])

        def main_pass(_iv=None):
            # emission order tuned for in-order engine queues: PE sees
            # [attn(s), wo(s), LN1transp(s+1), qkv(s+1), LN2transp(s), ffn(s)]
            # so LN chains hide behind independent matmul work
            x_cur = load_x(0)
            h_cur = ln1_full(x_cur)
            qkv_cur = qkv_phase(h_cur)
            exps_cur = attn_scores(qkv_cur[0], qkv_cur[1])
            for s in range(NSUP):
                if s + 1 < NSUP:
                    # x(s+1) stats depend only on the DMA: run the DVE chain
                    # under attention(s) so LN1-normalize never gates PE
                    x_next = load_x(s + 1)
                    st1 = ln_stats(x_next)
                attn_fm = attention_phase(exps_cur, qkv_cur[2])
                o1_ts, ob_ts = wo_phase(attn_fm, x_cur)
                st2 = ln_stats(o1_ts)
                if s + 1 < NSUP:
                    h_next = hpool.tile([P, CJ, TS], BF16, tag="hfm")
                    ln_apply(x_next, st1, h_next)
                    qkv_next = qkv_phase(h_next)
                h2_fm = h2pool.tile([P, CJ, TS], FP8, tag="h2fm")
                ln_apply(o1_ts, st2, h2_fm)
                if s + 1 < NSUP:
                    # next supertile's scores+exps stream on Act while the
                    # FFN below owns PE
                    exps_next = attn_scores(qkv_next[0], qkv_next[1])
                ffn_phase(h2_fm, ob_ts, s * TS)
                if s + 1 < NSUP:
                    x_cur, qkv_cur, exps_cur = x_next, qkv_next, exps_next

        if niter == 1:
            main_pass()
        else:
            with tc.For_i(0, niter, 1) as iv:
                main_pass(iv)

    nc.compile()
    return nc


def _build_runner(nc):
    """Reusable multi-core PJRT executor (mirrors bass_utils' axon path)."""
    import jax
    from jax.sharding import Mesh, PartitionSpec
    from jax.experimental.shard_map import shard_map
    import concourse.mybir as mybir
    from concourse.bass2jax import (install_neuronx_cc_hook, _bass_exec_p,
                                    partition_id_tensor)

    install_neuronx_cc_hook()
    partition_name = (nc.partition_id_tensor.name
                      if nc.partition_id_tensor else None)
    in_names, out_names, out_avals = [], [], []
    for alloc in nc.m.functions[0].allocations:
        if not isinstance(alloc, mybir.MemoryLocationSet):
            continue
        name = alloc.memorylocations[0].name
        if alloc.kind == "ExternalInput":
            if name != partition_name:
                in_names.append(name)
        elif alloc.kind == "ExternalOutput":
            out_names.append(name)
            out_avals.append(jax.core.ShapedArray(
                tuple(alloc.tensor_shape), mybir.dt.np(alloc.dtype)))
    n_params = len(in_names)
    all_in_names = list(in_names) + list(out_names)
    if partition_name is not None:
        all_in_names.append(partition_name)

    def _body(*args):
        operands = list(args)
        if partition_name is not None:
            operands.append(partition_id_tensor())
        outs = _bass_exec_p.bind(
            *operands,
            out_avals=tuple(out_avals),
            in_names=tuple(all_in_names),
            out_names=tuple(out_names),
            lowering_input_output_aliases=(),
            sim_require_finite=True,
            sim_require_nnan=True,
            nc=nc,
        )
        return tuple(outs)

    devices = jax.devices()[:NCORES]
    mesh = Mesh(np.asarray(devices), ("core",))
    n_outs = len(out_names)
    sharded = jax.jit(
        shard_map(_body, mesh=mesh,
                  in_specs=(PartitionSpec("core"),) * (n_params + n_outs),
                  out_specs=(PartitionSpec("core"),) * n_outs,
                  check_rep=False),
        keep_unused=True,
    )
    return sharded, in_names, out_names, out_avals


def _get_exec(niter=1):
    key = niter
    if key not in _CACHE:
        nc = _build_nc(niter)
        _CACHE[key] = _build_runner(nc)
    return _CACHE[key]


def _run(in_maps, niter=1):
    import jax
    sharded, in_names, out_names, out_avals = _get_exec(niter)
    concat_in = [np.concatenate([np.asarray(in_maps[c][n])
                                 for c in range(NCORES)], axis=0)
                 for n in in_names]
    concat_zeros = [np.zeros((NCORES * av.shape[0], *av.shape[1:]), av.dtype)
                    for av in out_avals]
    out_arrs = sharded(*concat_in, *concat_zeros)
    jax.block_until_ready(out_arrs)
    res = np.asarray(out_arrs[out_names.index("out")])
    return res.reshape(NCORES, NTOK, C)


def _make_in_maps(inputs):
    x = np.ascontiguousarray(np.asarray(inputs["x"], dtype=np.float32))
    reps = {k: np.ascontiguousarray(np.asarray(v, dtype=np.float32))
            for k, v in inputs.items() if k != "x"}
    in_maps = []
    for c in range(NCORES):
        m = dict(reps)
        m["x"] = x[c * B_LOC:(c + 1) * B_LOC].reshape(NTOK, C)
        in_maps.append(m)
    return in_maps


def kernel(**inputs) -> np.ndarray:
    in_maps = _make_in_maps(inputs)
    res = _run(in_maps, niter=1)
    return res.reshape(B, T, C)


def bench(inputs, niter=513, reps=5, floor_ns=72_400_000):
    """Estimate per-pass HW time by running the niter-looped build and
    subtracting the axon per-call RPC floor."""
    import time, jax
    in_maps = _make_in_maps(inputs)
    sharded, in_names, out_names, out_avals = _get_exec(niter)
    concat_in = [np.concatenate([np.asarray(in_maps[c][n])
                                 for c in range(NCORES)], axis=0)
                 for n in in_names]
    concat_zeros = [np.zeros((NCORES * av.shape[0], *av.shape[1:]), av.dtype)
                    for av in out_avals]
    import jax as _jax
    dev_in = [_jax.device_put(a) for a in concat_in]
    dev_zeros = [_jax.device_put(a) for a in concat_zeros]
    out = sharded(*dev_in, *dev_zeros)
    _jax.block_until_ready(out)  # compile + warm
    times = []
    for _ in range(reps):
        t0 = time.perf_counter()
        out = sharded(*dev_in, *dev_zeros)
        _jax.block_until_ready(out)
        times.append(time.perf_counter() - t0)
    res = np.asarray(out[out_names.index("out")]).reshape(NCORES, NTOK, C)
    wall_ns = np.array(times) * 1e9
    per_pass = (wall_ns - floor_ns) / niter
    return res.reshape(B, T, C), per_pass, wall_ns
